# revision 17
# baseline (speedup 1.0000x reference)
"""Causal self-attention (B=2, T=2048, C=1024, H=16, D=64) on 8 trn2 NeuronCores.

Sharding: core c = (batch b = c//4) x (head-group g = c%4; heads 4g..4g+3).
Tensor-parallel on heads for qkv_proj (column split) / out_proj (row split),
data-parallel on batch. Each core computes a full [C, T] partial of the
output projection for its batch; the host sums the 4 head-group partials
per batch and transposes back to [T, C].

Device kernel (per core), all matmul operands bf16 (psum accumulate f32).
Software-pipelined over tq chunks of 512 with cross-phase interleaving:
the attention t-loop of chunk cq is the spine; qkv-projection units of
chunk cq+1 and out-projection units of chunk cq-1 are issued as PE filler
inside it so the PE never idles (keeps the HAM clock-gate warm) while ACT
works through the exp stream.

Layout choices:
  - Q^T/K^T via W-stationary matmuls -> qk_sb [d, T]; psum evac + bias on
    ACT (which otherwise only runs exp).
  - V computed directly in [t, d] layout via x^T-stationary matmuls (no
    PE transposes); bias from a host-replicated tile, fused into one DVE
    evacuation per t-tile.
  - va_sb per (t-tile, head): [ones64 | V_h] (128 cols). PV lhsT is the
    whole block, M=128: the softmax denominator l lands REPLICATED on
    psum rows 0:64 and U^T on rows 64:128 for BOTH heads of a pair, so
    normalize is one reciprocal_approx_fast straight off psum rows 0:64
    (both heads at once), one partition-shift DMA of the reciprocal, and
    two DVE muls. un rows are [U_odd; U_even]; W_out rows are permuted
    on the host to match, which costs nothing.
  - S^T per (pair, t-tile) into [128, 2*512] f32 psum (two heads in PE
    row strips, concurrently); one 3D-AP exp per pair amortizes ACT
    instruction overhead.
  - PSUM: 2x [128,1024] S slots shared round-robin with the qkv/out-proj
    filler units + 2x [128,1024] U accumulators = 8 banks.
  - Causal masks on gpsimd (SBUF-only engine; it cannot touch PSUM).
"""

import sys

if "/opt/trn_rl_repo" not in sys.path:
    sys.path.insert(0, "/opt/trn_rl_repo")

import numpy as np

B, T, C = 2, 2048, 1024
H, D = 16, 64
HPC = 4            # heads per core
NC_ = HPC * D      # 256 qkv columns per core per projection
N_CORES = 8
PT = 128           # partition tile
TT = T // PT       # 16 t tiles
QC = 512           # tq chunk (moving free dim)
NQC = T // QC      # 4 tq chunks
KC = C // PT       # 8 contraction chunks for qkv proj
HB = 2 * D         # 128 cols per head block in va_sb ([ones|V])

_CACHE = {}


def _build_nc():
    import concourse.bacc as bacc
    import concourse.mybir as mybir
    import concourse.tile as tile
    from contextlib import ExitStack

    f32 = mybir.dt.float32
    bf16 = mybir.dt.bfloat16
    Act = mybir.ActivationFunctionType

    nc = bacc.Bacc("TRN2", target_bir_lowering=False, debug=False,
                   num_devices=N_CORES)

    xT_d = nc.dram_tensor("xT", [C, T], bf16, kind="ExternalInput").ap()
    wqkv_d = nc.dram_tensor("wqkv", [C, 3 * NC_], bf16, kind="ExternalInput").ap()
    bq_d = nc.dram_tensor("bq", [PT, 4], f32, kind="ExternalInput").ap()
    bvrep_d = nc.dram_tensor("bvrep", [PT, NC_], f32, kind="ExternalInput").ap()
    wout_d = nc.dram_tensor("wout", [NC_, C], bf16, kind="ExternalInput").ap()
    bout_d = nc.dram_tensor("bout", [PT, C // PT], f32, kind="ExternalInput").ap()
    trimask_d = nc.dram_tensor("trimask", [PT, PT], bf16, kind="ExternalInput").ap()
    outT_d = nc.dram_tensor("outT", [C, T], bf16, kind="ExternalOutput").ap()

    with tile.TileContext(nc) as tc, ExitStack() as ctx:
        p_xt = ctx.enter_context(tc.tile_pool(name="xt", bufs=1))
        p_wq = ctx.enter_context(tc.tile_pool(name="wq", bufs=1))
        p_qk = ctx.enter_context(tc.tile_pool(name="qk", bufs=4))
        p_va = ctx.enter_context(tc.tile_pool(name="va", bufs=TT))
        p_wo = ctx.enter_context(tc.tile_pool(name="wo", bufs=2))
        p_un = ctx.enter_context(tc.tile_pool(name="un", bufs=2 * NQC))
        p_small = ctx.enter_context(tc.tile_pool(name="small", bufs=1))
        p_pt = ctx.enter_context(tc.tile_pool(name="ptile", bufs=4))
        p_norm = ctx.enter_context(tc.tile_pool(name="norm", bufs=2))
        p_out = ctx.enter_context(tc.tile_pool(name="outs", bufs=4))
        ps_s = ctx.enter_context(tc.tile_pool(name="pss", bufs=2, space="PSUM"))
        ps_u = ctx.enter_context(tc.tile_pool(name="psu", bufs=2, space="PSUM"))

        # ---- loads. Big strided DMAs (half the k-range per trigger) keep
        # trigger-queue time off the critical path: wqkv+wout on sync,
        # x chunk 0 on gpsimd, x rest + small constants on scalar (all
        # landed before the first exp reaches ACT).
        wq_big = p_wq.tile([PT, KC * 3 * NC_], bf16, tag="wq")
        wq_sb = [wq_big[:, k * 3 * NC_:(k + 1) * 3 * NC_] for k in range(KC)]
        wqv = wq_big.rearrange("p (k c) -> p k c", c=3 * NC_)
        wqs = wqkv_d.rearrange("(k p) c -> p k c", p=PT)
        for h in range(2):
            ks = slice(4 * h, 4 * h + 4)
            nc.sync.dma_start(wqv[:, ks, :], wqs[:, ks, :])

        xt_big = p_xt.tile([PT, KC * T], bf16, tag="xt")
        xt_sb = [xt_big[:, k * T:(k + 1) * T] for k in range(KC)]
        xtv = xt_big.rearrange("p (k t) -> p k t", t=T)
        xts = xT_d.rearrange("(k p) t -> p k t", p=PT)
        for h in range(2):
            ks = slice(4 * h, 4 * h + 4)
            nc.gpsimd.dma_start(xtv[:, ks, 0:QC], xts[:, ks, 0:QC])

        bq_sb = p_small.tile([PT, 4], f32, tag="bq")
        nc.scalar.dma_start(bq_sb[:], bq_d[:])
        trimask = p_small.tile([PT, PT], bf16, tag="trimask")
        nc.scalar.dma_start(trimask[:], trimask_d[:])
        # x chunk 1 now; chunks 2-3 deferred past the prologue so the
        # critical chunk-0 loads aren't starved of HBM bandwidth.
        for h in range(2):
            ks = slice(4 * h, 4 * h + 4)
            nc.scalar.dma_start(xtv[:, ks, QC:2 * QC], xts[:, ks, QC:2 * QC])
        bvrep = p_small.tile([PT, NC_], f32, tag="bvrep")
        nc.scalar.dma_start(bvrep[:], bvrep_d[:])
        bout_sb = p_small.tile([PT, C // PT], f32, tag="bout")
        nc.scalar.dma_start(bout_sb[:], bout_d[:])

        wo_sb = []
        for k in range(2):
            w = p_wo.tile([PT, C], bf16, tag="wo")
            nc.sync.dma_start(w[:], wout_d[k * PT:(k + 1) * PT, :])
            wo_sb.append(w)

        qk_sb = [p_qk.tile([PT, T], bf16, tag="qk", name=f"qk{j}") for j in range(4)]
        va_sb = [p_va.tile([PT, 4 * HB], bf16, tag="va", name=f"va{t}")
                 for t in range(TT)]
        un_sb = [[p_un.tile([PT, QC], bf16, tag="un", name=f"un{j}_{c}")
                  for c in range(NQC)] for j in range(2)]

        # ones blocks (cols 0:64 of each [ones|V] head block)
        for t in range(TT):
            nc.vector.memset(
                va_sb[t].rearrange("p (h c) -> p h c", c=HB)[:, :, 0:D],
                1.0)

        # ------------------------------------------------- work units --
        def qk_unit(cq, m):
            # Q^T/K^T columns for chunk cq, projection block m (0,1=Q 2,3=K)
            cs = slice(cq * QC, (cq + 1) * QC)
            ps = ps_s.tile([PT, 2 * QC], f32, tag="sq")
            for k in range(KC):
                nc.tensor.matmul(
                    ps[:, 0:QC], wq_sb[k][:, m * PT:(m + 1) * PT],
                    xt_sb[k][:, cs],
                    start=(k == 0), stop=(k == KC - 1),
                )
            nc.scalar.activation(qk_sb[m][:, cs], ps[:, 0:QC], Act.Identity,
                                 bias=bq_sb[:, m:m + 1])

        def v_unit(cq, t4):
            # V[t, d] for t-tile t = 4*cq + t4, all 4 heads, x^T-stationary
            t = 4 * cq + t4
            tsl = slice(t * PT, (t + 1) * PT)
            ps = ps_s.tile([PT, 2 * QC], f32, tag="sq")
            for k in range(KC):
                nc.tensor.matmul(
                    ps[:, 0:NC_], xt_sb[k][:, tsl], wq_sb[k][:, 2 * NC_:3 * NC_],
                    start=(k == 0), stop=(k == KC - 1),
                )
            ps4 = ps[:, 0:NC_].rearrange("p (h v) -> p h v", v=D)
            bv4 = bvrep.rearrange("p (h v) -> p h v", v=D)
            va4 = va_sb[t].rearrange("p (h c) -> p h c", c=HB)
            nc.vector.tensor_add(va4[:, :, D:HB], ps4[:, :, :], bv4[:, :, :])

        def op_unit(cq, e, act_evac=False):
            # out-proj rows e*128:(e+1)*128 for chunk cq
            cs = slice(cq * QC, (cq + 1) * QC)
            ps = ps_s.tile([PT, 2 * QC], f32, tag="sq")
            for k in range(2):
                nc.tensor.matmul(
                    ps[:, 0:QC], wo_sb[k][:, e * PT:(e + 1) * PT],
                    un_sb[k][cq][:],
                    start=(k == 0), stop=(k == 1),
                )
            ot = p_out.tile([PT, QC], bf16, tag="ot")
            if act_evac:
                nc.scalar.activation(ot[:], ps[:, 0:QC], Act.Identity,
                                     bias=bout_sb[:, e:e + 1])
            else:
                nc.vector.tensor_scalar_add(ot[:], ps[:, 0:QC],
                                            bout_sb[:, e:e + 1])
            nc.sync.dma_start(outT_d[e * PT:(e + 1) * PT, cs], ot[:])

        # ------------------------------------------------- attention ---
        def attn_window(cq, fillers):
            nts = 4 * cq + 4
            psu = [ps_u.tile([PT, 2 * QC], f32, tag="u", name=f"u{cq}_{j}")
                   for j in range(2)]

            def s_tiles(t):
                # S^T [t-tile, q] per pair; two heads in concurrent PE
                # row strips, into the two banks of one psum slot.
                p = t - 4 * cq
                s = max(p, 0) * PT
                tsl = slice(t * PT, (t + 1) * PT)
                qsl = slice(cq * QC + s, (cq + 1) * QC)
                out = []
                for j in range(2):
                    psS = ps_s.tile([PT, 2 * QC], f32, tag="sq")
                    nc.tensor.matmul(
                        psS[:, s:QC],
                        qk_sb[2 + j][0:D, tsl], qk_sb[j][0:D, qsl],
                        start=True, stop=True, tile_position=(0, 0),
                    )
                    nc.tensor.matmul(
                        psS[:, QC + s:2 * QC],
                        qk_sb[2 + j][D:PT, tsl], qk_sb[j][D:PT, qsl],
                        start=True, stop=True, tile_position=(D, 0),
                    )
                    out.append(psS)
                return out

            cur = s_tiles(0)
            nf = len(fillers)
            fi = 0
            for t in range(nts):
                want = ((t + 1) * nf + nts - 1) // nts
                while fi < min(want, nf):
                    fillers[fi]()
                    fi += 1
                p = t - 4 * cq
                s = max(p, 0) * PT
                pts = []
                for j in range(2):
                    pt = p_pt.tile([PT, 2 * QC], bf16, tag="pt")
                    pt3 = pt.rearrange("p (h w) -> p h w", h=2)
                    ps3 = cur[j].rearrange("p (h w) -> p h w", h=2)
                    nc.scalar.activation(pt3[:, :, s:QC], ps3[:, :, s:QC],
                                         Act.Exp, scale=0.125)
                    pts.append(pt)
                if p >= 0:
                    for j in range(2):
                        nc.gpsimd.tensor_mul(
                            pts[j][:, s:s + PT], pts[j][:, s:s + PT],
                            trimask[:])
                        nc.gpsimd.tensor_mul(
                            pts[j][:, QC + s:QC + s + PT],
                            pts[j][:, QC + s:QC + s + PT], trimask[:])
                va4 = va_sb[t].rearrange("p (h c) -> p h c", c=HB)
                for j in range(2):
                    lhs_e = va4[:, 2 * j, :]      # [ones | V_even]
                    lhs_o = va4[:, 2 * j + 1, :]  # [ones | V_odd]
                    if p < 0:
                        nc.tensor.matmul(psu[j][:, 0:QC], lhs_e,
                                         pts[j][:, 0:QC],
                                         start=(t == 0), stop=False)
                        nc.tensor.matmul(psu[j][:, QC:2 * QC], lhs_o,
                                         pts[j][:, QC:2 * QC],
                                         start=(t == 0), stop=False)
                    elif s + PT < QC:
                        nc.tensor.matmul(psu[j][:, s + PT:QC], lhs_e,
                                         pts[j][:, s + PT:QC],
                                         start=(t == 0), stop=False)
                        nc.tensor.matmul(psu[j][:, QC + s + PT:2 * QC], lhs_o,
                                         pts[j][:, QC + s + PT:2 * QC],
                                         start=(t == 0), stop=False)
                for j in range(2):
                    if p >= 0:
                        # masked diagonal block. start=False even at t==0:
                        # the clean-part matmul already opened the bank and
                        # fresh has_written bits make this a plain write.
                        va4j = va_sb[t].rearrange("p (h c) -> p h c", c=HB)
                        nc.tensor.matmul(psu[j][:, s:s + PT],
                                         va4j[:, 2 * j, :],
                                         pts[j][:, s:s + PT],
                                         start=False, stop=(t == nts - 1))
                        nc.tensor.matmul(psu[j][:, QC + s:QC + s + PT],
                                         va4j[:, 2 * j + 1, :],
                                         pts[j][:, QC + s:QC + s + PT],
                                         start=False, stop=(t == nts - 1))
                    if t == nts - 1:
                        normalize(cq, j, psu[j])
                if t + 1 < nts:
                    cur = s_tiles(t + 1)
            while fi < nf:
                fillers[fi]()
                fi += 1

        def normalize(cq, j, psu):
            # l (both heads) replicated on psum rows 0:64; U on 64:128.
            # One reciprocal straight off psum, one partition-shift DMA,
            # two muls. un rows: [U_odd; U_even] (wout rows permuted on
            # the host to match).
            rb = p_norm.tile([D, 2 * QC], f32, tag="rb")
            nc.vector.reciprocal_approx_fast(rb[:], psu[0:D, :])
            rb2 = p_norm.tile([PT, 2 * QC], f32, tag="rb2")
            nc.scalar.dma_start(rb2[D:PT, :], rb[:])
            nc.vector.tensor_mul(un_sb[j][cq][D:PT, :],
                                 psu[D:PT, 0:QC], rb2[D:PT, 0:QC])
            ut = p_norm.tile([PT, QC], bf16, tag="ut")
            nc.vector.tensor_mul(ut[D:PT, :],
                                 psu[D:PT, QC:2 * QC], rb2[D:PT, QC:2 * QC])
            nc.scalar.dma_start(un_sb[j][cq][0:D, :], ut[D:PT, :])

        # ------------------------------------------------- schedule ----
        # chunk 0 prologue: its own qkv (deadlock-safe order: all qk units
        # before the attention S prologue), V tile 0; V tiles 1-3 become
        # in-loop filler together with chunk 1's qkv.
        for m in (0, 2, 1, 3):
            qk_unit(0, m)
        v_unit(0, 0)
        for h in range(2):
            ks = slice(4 * h, 4 * h + 4)
            nc.scalar.dma_start(xtv[:, ks, 2 * QC:T], xts[:, ks, 2 * QC:T])

        def qkv_units(cq):
            u = [lambda m=m: qk_unit(cq, m) for m in (0, 2, 1, 3)]
            u += [lambda t4=t4: v_unit(cq, t4) for t4 in range(4)]
            return u

        def op_units(cq, acts=()):
            return [lambda e=e: op_unit(cq, e, act_evac=(e in acts))
                    for e in range(C // PT)]

        def interleave(a, b):
            out = []
            for i in range(max(len(a), len(b))):
                if i < len(a):
                    out.append(a[i])
                if i < len(b):
                    out.append(b[i])
            return out

        f0 = interleave([lambda t4=t4: v_unit(0, t4) for t4 in (1, 2, 3)],
                        qkv_units(1))
        attn_window(0, f0)
        # op(0) evacs alternate ACT/DVE (ACT has slack in window 1);
        # window 2 is PE-bound so it carries only qkv(3); window 3 is
        # ACT-bound (exp-paced) so op(1) + half of op(2) ride as free PE
        # filler there (first two on ACT to dodge the boundary's DVE
        # congestion from the normalize muls); the rest of op(2) fills
        # the PE during window 3's normalize chain, then op(3) drains
        # with alternating evacs (ACT idles in the tail).
        attn_window(1, interleave(qkv_units(2), op_units(0, acts=(0, 2, 4, 6))))
        attn_window(2, qkv_units(3))
        op1 = op_units(1, acts=(0, 2))
        op2 = op_units(2, acts=(4, 6))
        attn_window(3, interleave(op1, op2[:4]))
        for u in op2[4:]:
            u()
        for u in op_units(3, acts=(1, 3, 5, 7)):
            u()

    nc.compile()
    return nc


def _get_nc():
    if "nc" not in _CACHE:
        _CACHE["nc"] = _build_nc()
    return _CACHE["nc"]


def _make_in_maps(x, W_qkv, b_qkv, W_out, b_out):
    import ml_dtypes

    bf16 = ml_dtypes.bfloat16
    x = np.asarray(x, dtype=np.float32)
    W_qkv = np.asarray(W_qkv, dtype=np.float32)
    b_qkv = np.asarray(b_qkv, dtype=np.float32)
    W_out = np.asarray(W_out, dtype=np.float32)
    b_out = np.asarray(b_out, dtype=np.float32)

    i = np.arange(PT)[:, None]
    j = np.arange(PT)[None, :]
    trimask = (i <= j).astype(bf16)

    # un rows per pair are [U_odd(0:64); U_even(64:128)] -> permute wout
    # rows to [head 2j+1 dims, head 2j dims] per pair j.
    perm = np.concatenate([
        np.concatenate([np.arange((2 * j + 1) * D, (2 * j + 2) * D),
                        np.arange(2 * j * D, (2 * j + 1) * D)])
        for j in range(2)
    ])  # within one 256-row head-group block

    in_maps = []
    for c in range(N_CORES):
        b, g = divmod(c, 4)
        gs = slice(g * NC_, (g + 1) * NC_)
        wqkv_c = np.ascontiguousarray(np.concatenate(
            [W_qkv[:, gs], W_qkv[:, C:][:, gs], W_qkv[:, 2 * C:][:, gs]],
            axis=1).astype(bf16))
        bqk_c = np.concatenate([b_qkv[gs], b_qkv[C:][gs]])      # [512] Q,K
        bv_c = b_qkv[2 * C:][gs]                                # [256] V
        bq_np = np.ascontiguousarray(bqk_c.reshape(4, PT).T.astype(np.float32))
        bvrep = np.ascontiguousarray(
            np.broadcast_to(bv_c[None, :], (PT, NC_)).astype(np.float32))
        bout_full = (b_out if g == 0 else np.zeros_like(b_out))
        bout_np = np.ascontiguousarray(
            bout_full.reshape(C // PT, PT).T.astype(np.float32))
        wout_c = W_out[gs, :][perm, :]
        in_maps.append({
            "xT": np.ascontiguousarray(x[b].T.astype(bf16)),
            "wqkv": wqkv_c,
            "bq": bq_np,
            "bvrep": bvrep,
            "wout": np.ascontiguousarray(wout_c.astype(bf16)),
            "bout": bout_np,
            "trimask": trimask,
        })
    return in_maps


def _assemble(results):
    out = np.empty((B, T, C), dtype=np.float32)
    for b in range(B):
        acc = results[4 * b]["outT"].astype(np.float32)
        for g in range(1, 4):
            acc += results[4 * b + g]["outT"].astype(np.float32)
        out[b] = acc.T
    return out


def kernel(x, W_qkv, b_qkv, W_out, b_out):
    from concourse import bass_utils
    nc = _get_nc()
    in_maps = _make_in_maps(x, W_qkv, b_qkv, W_out, b_out)
    res = bass_utils.run_bass_kernel_spmd(nc, in_maps, core_ids=list(range(N_CORES)))
    return _assemble(res.results)
